# revision 49
# baseline (speedup 1.0000x reference)
"""Multi-head attention forward on 8 Trainium2 NeuronCores (Bass/Tile).

Problem: x[4, 2048, 768] -> qkv proj (w_qkv[2304, 768]) -> 12-head attention
(softmax((q k^T) * 768^-0.5)) -> out proj (w_out[768, 768]).

Sharding: core c handles batch b = c//2 and a group of 6 heads g = c%2
(tensor parallel over heads within a batch pair). Each core computes a
partial output (its heads' contribution through the row-sliced out
projection, transposed: [768, 2048]); the host sums the two partials per
batch, transposes back and adds b_out.

Device-side layout notes (everything transposed so the contraction dim sits
on SBUF partitions):
  xT   [768, 2048]  loaded pre-transposed from the host (bf16)
  qkvT [feat, 2048] = wT.T @ xT via bf16 matmuls
  scoresT[keys, q]  = kT_tile.T @ qT  (so attn@v needs no transpose)
  softmax without max-subtraction (scores are O(1); exp is safe in fp32);
  denominator comes free from an appended ones-column in v ("v_aug"):
  j0 puts values on PSUM partitions 0:64 / denom on 64, j1 the reverse
  (ones at vaug col 128) so both halves' divides write attn_outT rows
  directly. 1/denom via DVE reciprocal_approx_fast (must run from
  partition base 0), rounded to f32r, PE-broadcast across partitions,
  divide folded into the PSUM->SBUF copyback on DVE. A burst of dummy
  identity matmuls right after the preamble opens the PE's HAM clock
  gate (1.2->2.4 GHz) before the first real chains.
"""

import os
import sys

import ml_dtypes
import numpy as np

_bf16 = ml_dtypes.bfloat16

if "/opt/trn_rl_repo" not in sys.path:
    sys.path.insert(0, "/opt/trn_rl_repo")

B = 4
N = 2048
DIM = 768
HEADS = 12
DHEAD = 64
SCALE = DIM ** (-0.5)
NCORES = 8
HPC = 6  # heads per core
FEAT = HPC * DHEAD  # 384 per-core attention features

_PROGRAM = None  # (nc,) cached compiled bass program


def _build_program():
    from contextlib import ExitStack

    import concourse.bass as bass
    import concourse.tile as tile
    from concourse import bacc, mybir
    from concourse.masks import make_identity

    f32 = mybir.dt.float32
    f32r = mybir.dt.float32r
    bf16 = mybir.dt.bfloat16
    f8 = mybir.dt.float8e4
    DRmode = mybir.MatmulPerfMode.DoubleRow
    Alu = mybir.AluOpType
    ActF = mybir.ActivationFunctionType

    nc = bacc.Bacc("TRN2", target_bir_lowering=False, debug=False)

    xT_in = nc.dram_tensor("xT", [DIM, N], bf16, kind="ExternalInput")
    wqkvT = nc.dram_tensor("wqkvT", [DIM, 3 * FEAT], bf16, kind="ExternalInput")
    bqkv = nc.dram_tensor("bqkv", [128, 9], f32, kind="ExternalInput")
    woutT = nc.dram_tensor("woutT", [FEAT, DIM], f32r, kind="ExternalInput")
    out_T = nc.dram_tensor("outT", [DIM, N], bf16, kind="ExternalOutput")

    NT = N // 128  # 16 n-tiles
    KC = DIM // 128  # 6 contraction chunks for dim
    NSPAN = N // 512  # 4 moving spans

    with tile.TileContext(nc) as tc, ExitStack() as ctx:
        const = ctx.enter_context(tc.tile_pool(name="const", bufs=1))
        identity_bf = const.tile([128, 128], bf16)
        make_identity(nc, identity_bf)
        ones_f32 = const.tile([128, 1], f32)
        nc.vector.memset(ones_f32[:, :], 1.0)
        ones65 = const.tile([65, 128], f32r)
        nc.vector.tensor_copy(
            out=ones65[:, :], in_=ones_f32[0:65, :].to_broadcast((65, 128))
        )
        bias_sb = const.tile([128, 9], f32)

        # ---- Phase 1: staged loads ordered by first-consumer ----
        # chain order consumes w cols m*128 for m = 0 (q), 3 (k), 6 (v) of
        # pair 0 first, and xT spans left-to-right; load those slices first,
        # fanned across the gpsimd/sync/vector DMA queues.
        xt_pool = ctx.enter_context(tc.tile_pool(name="xT", bufs=1))
        xT = xt_pool.tile([128, KC, N], bf16)
        wpool = ctx.enter_context(tc.tile_pool(name="w", bufs=1))
        w_all = wpool.tile([128, KC, 3 * FEAT], bf16)

        def ld_w(eng, j, c0, c1):
            eng.dma_start(w_all[:, j, c0:c1], wqkvT[j * 128 : (j + 1) * 128, c0:c1])

        def ld_x(eng, j, c0, c1):
            eng.dma_start(xT[:, j, c0:c1], xT_in[j * 128 : (j + 1) * 128, c0:c1])

        nc.scalar.dma_start(bias_sb[:, :], bqkv[:, :])
        for j in range(KC):
            ld_x(nc.gpsimd, j, 0, 512)
            ld_w(nc.sync, j, 0, 128)
            ld_w(nc.scalar, j, 768, 896)
        for j in range(KC):
            ld_x(nc.gpsimd, j, 512, 1024)
            ld_w(nc.sync, j, 384, 512)
            ld_w(nc.scalar, j, 896, 1152)
        for j in range(KC):
            ld_x(nc.gpsimd, j, 1024, 1536)
            ld_x(nc.sync, j, 1536, 2048)
        for j in range(KC):
            ld_w(nc.sync, j, 128, 384)
        for j in range(KC):
            ld_w(nc.sync, j, 512, 768)
        wout_sb = wpool.tile([128, 3, DIM], f32r)
        for c in range(3):
            nc.sync.dma_start(wout_sb[:, c, :], woutT[c * 128 : (c + 1) * 128, :])

        # PSUM: spool 3x[128,2,512] (6 banks) + opool 2x[128,512] (2 banks)
        spool = ctx.enter_context(tc.tile_pool(name="spsum", bufs=3, space="PSUM"))
        opool = ctx.enter_context(tc.tile_pool(name="opsum", bufs=2, space="PSUM"))

        # warm the PE's HAM clock-gate while the first DMAs land: ~3us of
        # dummy matmuls on the identity tile move the 1.2->2.4GHz transition
        # ahead of the first real chains
        warm = spool.tile([128, 2, 512], f32, tag="s", name="warm")
        for _ in range(28):
            nc.tensor.matmul(
                warm[:, 0, 0:128],
                identity_bf[:, :],
                identity_bf[:, :],
                start=True,
                stop=True,
            )

        qk_pool = ctx.enter_context(tc.tile_pool(name="qk", bufs=2))
        va_pool = ctx.enter_context(tc.tile_pool(name="vaug", bufs=1))
        # v for ALL pairs, produced directly in [keys, vfeat] layout (no PE
        # transposes): per kc block and pair hp:
        #   j0 = [vA@0:64 | ones@64 | 0]     -> attn@v values on partitions
        #        0:64, denom on 64
        #   j1 = [ones@128 | 0 | vB@192:256] -> values on partitions 64:128,
        #        denom on 0
        vaug_all = va_pool.tile([128, NT, 3, 256], bf16)
        exp_pool = ctx.enter_context(tc.tile_pool(name="expT", bufs=6))
        rcp_pool = ctx.enter_context(tc.tile_pool(name="rcp", bufs=2))
        sbo_pool = ctx.enter_context(tc.tile_pool(name="sbo", bufs=3))
        ao_pool = ctx.enter_context(tc.tile_pool(name="attnout", bufs=1))
        attn_outT = ao_pool.tile([128, 3, N], f32r)
        ost_pool = ctx.enter_context(tc.tile_pool(name="ostage", bufs=3))

        def emit_qk_chain(qk_t, hp, idx, m, sp2):
            """One (m, span-pair): 12 matmuls + biased copyback."""
            ps = spool.tile([128, 2, 512], f32, tag="s", name="ps_qkv")
            for u in range(2):
                span = 2 * sp2 + u
                for j in range(KC):
                    nc.tensor.matmul(
                        ps[:, u, :],
                        w_all[:, j, m * 128 : (m + 1) * 128],
                        xT[:, j, span * 512 : (span + 1) * 512],
                        start=(j == 0),
                        stop=(j == KC - 1),
                    )
            cols = slice(sp2 * 1024, (sp2 + 1) * 1024)
            # head A -> chunk idx rows 0:64; head B -> chunk idx+2 rows 64:128
            nc.vector.tensor_scalar(
                qk_t[0:64, idx, cols].rearrange("p (a b) -> p a b", a=2),
                ps[0:64, :, :],
                bias_sb[0:64, m : m + 1],
                None,
                Alu.add,
            )
            nc.vector.tensor_scalar(
                qk_t[64:128, idx + 2, cols].rearrange("p (a b) -> p a b", a=2),
                ps[64:128, :, :],
                bias_sb[64:128, m : m + 1],
                None,
                Alu.add,
            )

        def vinit():
            # pair 0's zeros go on the scalar queue (free early — gpsimd is
            # busy generating load descriptors) so the DVE ones-copies, which
            # sit at the head of the in-order DVE queue, unblock before the
            # first q/k copybacks need the DVE. v bias is folded into b_out
            # on the host (softmax rows sum to 1), so no bias add here.
            nc.scalar.memzero(vaug_all[:, :, 0, :])
            nc.vector.tensor_copy(
                out=vaug_all[:, :, 0, 64:65],
                in_=ones_f32[:, :].to_broadcast((128, NT, 1)),
            )
            nc.vector.tensor_copy(
                out=vaug_all[:, :, 0, 128:129],
                in_=ones_f32[:, :].to_broadcast((128, NT, 1)),
            )

        def vinit_rest():
            # pairs 1-2: not needed until their attn@v, well past startup
            nc.gpsimd.memset(vaug_all[:, :, 1:3, :], 0.0)
            nc.vector.tensor_copy(
                out=vaug_all[:, :, 1:3, 64:65],
                in_=ones_f32[:, :].to_broadcast((128, NT, 2, 1)),
            )
            nc.vector.tensor_copy(
                out=vaug_all[:, :, 1:3, 128:129],
                in_=ones_f32[:, :].to_broadcast((128, NT, 2, 1)),
            )

        def emit_vunit(kc):
            """v for one 128-key block, all 3 pairs: x.T @ w_v directly in
            [keys, vfeat] layout — no transposes needed."""
            pv = spool.tile([128, 384], f32, tag="s", name="pv")
            for j in range(KC):
                nc.tensor.matmul(
                    pv[:, :],
                    xT[:, j, kc * 128 : (kc + 1) * 128],
                    w_all[:, j, 768:1152],
                    start=(j == 0),
                    stop=(j == KC - 1),
                )
            pvv = pv[:, :].rearrange("p (h t c) -> p h t c", h=3, t=2)
            nc.vector.tensor_copy(out=vaug_all[:, kc, :, 0:64], in_=pvv[:, :, 0, :])
            nc.vector.tensor_copy(
                out=vaug_all[:, kc, :, 192:256], in_=pvv[:, :, 1, :]
            )

        def make_pair_units(hp):
            """Allocate q/k tile + return (tile, list of PE filler closures).

            No zero padding: the scores matmuls read each head's own 64
            partition rows (j0 at base 0, j1 at base 64 / tile position
            (64, 0)); the unused quadrants of qk_t are never addressed.
            """
            qk_t = qk_pool.tile([128, 4, N], bf16, name="qk", tag="qk")
            units = []
            # k chains (idx 1) for BOTH span-pairs before the second q chain:
            # the first score block contracts over all 2048 keys
            order = [
                (0, hp, 0),
                (1, 3 + hp, 0),
                (1, 3 + hp, 1),
                (0, hp, 1),
            ]
            for idx, m, sp2 in order:
                units.append(
                    lambda i=idx, mm=m, s=sp2: emit_qk_chain(qk_t, hp, i, mm, s)
                )
            return qk_t, units

        def emit_outproj(m, span):
            """One out-proj tile [128, 512]: 3 matmuls + copyback + DMA."""
            ps = spool.tile([128, 2, 512], f32, tag="s", name="ps_op")
            for c in range(3):
                nc.tensor.matmul(
                    ps[:, 0, :],
                    wout_sb[:, c, m * 128 : (m + 1) * 128],
                    attn_outT[:, c, span * 512 : (span + 1) * 512],
                    start=(c == 0),
                    stop=(c == 2),
                )
            ostage = ost_pool.tile([128, 512], bf16, name="ostage", tag="ostage")
            nc.vector.tensor_copy(out=ostage[:, :], in_=ps[:, 0, :])
            eng = (nc.gpsimd, nc.sync, nc.scalar)[m % 3]
            eng.dma_start(
                out_T[m * 128 : (m + 1) * 128, span * 512 : (span + 1) * 512],
                ostage[:, :],
            )

        # ---- phase 1: q/k for pair 0 (DMA-gated). v units are emitted
        # just-in-time inside pair 0's first span (2 per half) so each
        # vunit(kc) precedes the attn@v that reads it in program order ----
        vinit()
        vq = [lambda k=kc: emit_vunit(k) for kc in range(NT)]
        cur_qk, units0 = make_pair_units(0)

        # keep the PE busy through the DMA-gated pre chains so the HAM
        # clock-gate's 3.4us activity window never lapses (a lapse holds
        # the whole startup region at 1.2GHz)
        warm_ka = opool.tile([128, 512], f32, tag="o", name="warm_ka")

        def keepalive(n=8):
            for _ in range(n):
                nc.tensor.matmul(
                    warm_ka[:, 0:128],
                    identity_bf[:, :],
                    identity_bf[:, :],
                    start=True,
                    stop=True,
                )

        for u in units0[:3]:
            u()
            keepalive()
        vinit_rest()

        # ---- attention per head pair, interleaving filler PE work ----
        for hp in range(3):
            qk = cur_qk
            if hp < 2:
                cur_qk, nxt = make_pair_units(hp + 1)
                filler = (units0[3:] + nxt) if hp == 0 else nxt
                fill_stride = max(1, (64 + len(filler)) // (len(filler) + 1))
            else:
                filler = []  # outproj units appended dynamically by normalize
                fill_stride = 1
            half_ctr = 0
            pending = [None]  # deferred normalize closure

            def flush_pending():
                if pending[0] is not None:
                    pending[0]()
                    pending[0] = None

            for j in range(2):
                rows = slice(0, 64) if j == 0 else slice(64, 128)
                qT = qk[rows, 2 * j, :]
                kT = qk[rows, 2 * j + 1, :]
                for span in range(NSPAN):
                    po = opool.tile([128, 512], f32, tag="o", name="po")
                    ets = []
                    for half in range(8):
                        ps = spool.tile([128, 2, 512], f32, tag="s", name="ps_s")
                        for u in range(2):
                            kc = 2 * half + u
                            nc.tensor.matmul(
                                ps[:, u, :],
                                kT[:, kc * 128 : (kc + 1) * 128],
                                qT[:, span * 512 : (span + 1) * 512],
                                start=True,
                                stop=True,
                            )
                        et = exp_pool.tile([128, 2, 512], bf16)
                        nc.scalar.activation(
                            et[:, :, :], ps[:, :, :], ActF.Exp, scale=float(SCALE)
                        )
                        ets.append(et)
                        if half == 5:
                            # deferred so the DVE recip chain has finished —
                            # the PE broadcast below never stalls
                            flush_pending()
                        if vq:
                            # pair 0, first span only: keep vunit(kc) ahead
                            # of the attn@v that consumes it
                            vq.pop(0)()
                            if vq:
                                vq.pop(0)()
                        if half >= 1:
                            pet = ets[half - 1]
                            for u in range(2):
                                kc = 2 * (half - 1) + u
                                nc.tensor.matmul(
                                    po[:, :],
                                    vaug_all[:, kc, hp, j * 128 : (j + 1) * 128],
                                    pet[:, u, :],
                                    start=(kc == 0),
                                    stop=False,
                                )
                        half_ctr += 1
                        if hp == 2:
                            if len(filler) > 6:
                                filler.pop(0)()
                        elif filler and fill_stride and half_ctr % fill_stride == 0:
                            filler.pop(0)()
                    pet = ets[7]
                    for u in range(2):
                        kc = 14 + u
                        nc.tensor.matmul(
                            po[:, :],
                            vaug_all[:, kc, hp, j * 128 : (j + 1) * 128],
                            pet[:, u, :],
                            start=False,
                            stop=(kc == 15),
                        )
                    # denominator recip + PSUM->SBUF copy issued immediately
                    # so the deferred broadcast matmul never waits on DVE.
                    # j0: values on po[0:64], denom row 64; j1: denom row 0,
                    # values on po[64:128] (set by the vaug column layout) so
                    # both j write attn_outT partitions directly.
                    drow = 64 if j == 0 else 0
                    rsf = rcp_pool.tile([65, 512], f32, name="rsf", tag="rsf")
                    # NB: approx recip requires partition base 0 — rows other
                    # than drow compute garbage but are never read
                    nc.vector.reciprocal_approx_fast(rsf[0:65, :], po[0:65, :])
                    rs = rcp_pool.tile([65, 512], f32r, name="rs", tag="rs")
                    with nc.allow_low_precision(reason="fp32r round for PE bcast"):
                        nc.vector.tensor_copy(
                            out=rs[drow : drow + 1, :], in_=rsf[drow : drow + 1, :]
                        )
                    sb_o = sbo_pool.tile([128, 512], f32, name="sb_o")
                    vrows = slice(0, 64) if j == 0 else slice(64, 128)
                    nc.vector.tensor_copy(out=sb_o[vrows, :], in_=po[vrows, :])

                    def normalize(j=j, span=span, po=po, hp=hp, rs=rs, sb_o=sb_o):
                        # PE-broadcast of the recip back into po's own bank
                        # (WAR-ordered after the early copy), then divide.
                        cols = slice(span * 512, (span + 1) * 512)
                        if j == 0:
                            nc.tensor.matmul(
                                po[0:64, :],
                                ones65[64:65, 0:64],
                                rs[64:65, :],
                                start=True,
                                stop=True,
                            )
                            nc.vector.tensor_tensor(
                                out=attn_outT[0:64, hp, cols],
                                in0=sb_o[0:64, :],
                                in1=po[0:64, :],
                                op=Alu.mult,
                            )
                        else:
                            # dst must start at partition 0: broadcast the
                            # recip to all 128 partitions; rows 0:64 are junk
                            # but po's denom/zero rows are already consumed
                            nc.tensor.matmul(
                                po[:, :],
                                ones65[0:1, :],
                                rs[0:1, :],
                                start=True,
                                stop=True,
                            )
                            nc.vector.tensor_tensor(
                                out=attn_outT[64:128, hp, cols],
                                in0=sb_o[64:128, :],
                                in1=po[64:128, :],
                                op=Alu.mult,
                            )
                            if hp == 2:
                                for m in range(DIM // 128):
                                    filler.append(
                                        lambda mm=m, s=span: emit_outproj(mm, s)
                                    )

                    pending[0] = normalize
            flush_pending()
            while filler:
                filler.pop(0)()

    nc.compile()
    return nc


def _get_program():
    global _PROGRAM
    if _PROGRAM is None:
        _PROGRAM = _build_program()
    return _PROGRAM


def _round_to_f32r(a):
    """Round fp32 to the PE's fp32r format: 11-bit mantissa, low 12 bits zero
    (round to nearest, ties away handled approximately via +0x7FF + lsb)."""
    u = np.ascontiguousarray(a, dtype=np.float32).view(np.uint32)
    r = u + np.uint32(0x7FF) + ((u >> np.uint32(12)) & np.uint32(1))
    r &= np.uint32(0xFFFFF000)
    return r.view(np.float32)


def make_core_inputs(x, w_qkv, b_qkv, w_out):
    """Host-side shard: per-core input dicts for cores 0..7."""
    x = np.asarray(x, dtype=np.float32)
    w_qkv = np.asarray(w_qkv, dtype=np.float32)
    b_qkv = np.asarray(b_qkv, dtype=np.float32)
    w_out = np.asarray(w_out, dtype=np.float32)

    per_group = []
    for g in range(2):
        rows = np.concatenate(
            [
                w_qkv[qkv * DIM + g * FEAT : qkv * DIM + (g + 1) * FEAT]
                for qkv in range(3)
            ],
            axis=0,
        )  # [1152, 768]
        wqkvT_g = np.ascontiguousarray(rows.T).astype(_bf16)  # [768, 1152]
        b_rows = np.concatenate(
            [
                b_qkv[qkv * DIM + g * FEAT : qkv * DIM + (g + 1) * FEAT]
                for qkv in range(3)
            ],
            axis=0,
        )  # [1152]
        bias_g = np.ascontiguousarray(b_rows.reshape(9, 128).T)  # [128, 9]
        woutT_g = _round_to_f32r(w_out[:, g * FEAT : (g + 1) * FEAT].T)
        per_group.append((wqkvT_g, bias_g, woutT_g))

    xT_bf = [np.ascontiguousarray(x[b].T).astype(_bf16) for b in range(B)]
    in_maps = []
    for c in range(NCORES):
        b, g = c // 2, c % 2
        wqkvT_g, bias_g, woutT_g = per_group[g]
        in_maps.append(
            {
                "xT": xT_bf[b],
                "wqkvT": wqkvT_g,
                "bqkv": bias_g,
                "woutT": woutT_g,
            }
        )
    return in_maps


def assemble_output(results, b_out):
    """Host-side unshard: sum partials per batch pair, transpose, add bias."""
    b_out = np.asarray(b_out, dtype=np.float32)
    out = np.empty((B, N, DIM), dtype=np.float32)
    for b in range(B):
        pT = results[2 * b]["outT"].astype(np.float32) + results[2 * b + 1][
            "outT"
        ].astype(np.float32)  # [768, 2048]
        out[b] = pT.T + b_out[None, :]
    return out


def kernel(x, w_qkv, b_qkv, w_out, b_out):
    from concourse.bass_utils import run_bass_kernel_spmd

    nc = _get_program()
    in_maps = make_core_inputs(x, w_qkv, b_qkv, w_out)
    res = run_bass_kernel_spmd(nc, in_maps, list(range(NCORES)))
    # v bias is not applied on-device: softmax rows sum to 1, so it shifts
    # attn output by the constant b_v, i.e. adds w_out @ b_v to the output
    b_eff = np.asarray(b_out, np.float32) + np.asarray(w_out, np.float32) @ np.asarray(
        b_qkv, np.float32
    )[2 * DIM :]
    return assemble_output(res.results, b_eff)



# revision 50
# speedup vs baseline: 1.0508x; 1.0508x over previous
"""Multi-head attention forward on 8 Trainium2 NeuronCores (Bass/Tile).

Problem: x[4, 2048, 768] -> qkv proj (w_qkv[2304, 768]) -> 12-head attention
(softmax((q k^T) * 768^-0.5)) -> out proj (w_out[768, 768]).

Sharding: core c handles batch b = c//2 and a group of 6 heads g = c%2
(tensor parallel over heads within a batch pair). Each core computes a
partial output (its heads' contribution through the row-sliced out
projection, transposed: [768, 2048]); the host sums the two partials per
batch, transposes back and adds b_out.

Device-side layout notes (everything transposed so the contraction dim sits
on SBUF partitions):
  xT   [768, 2048]  loaded pre-transposed from the host (bf16)
  qkvT [feat, 2048] = wT.T @ xT via bf16 matmuls
  scoresT[keys, q]  = kT_tile.T @ qT  (so attn@v needs no transpose)
  softmax without max-subtraction (scores are O(1); exp is safe in fp32);
  denominator comes free from an appended ones-column in v ("v_aug"):
  j0 puts values on PSUM partitions 0:64 / denom on 64, j1 the reverse
  (ones at vaug col 128) so both halves' divides write attn_outT rows
  directly. 1/denom via DVE reciprocal_approx_fast (must run from
  partition base 0), rounded to f32r, PE-broadcast across partitions,
  divide folded into the PSUM->SBUF copyback on DVE. A burst of dummy
  identity matmuls right after the preamble opens the PE's HAM clock
  gate (1.2->2.4 GHz) before the first real chains.
"""

import os
import sys

import ml_dtypes
import numpy as np

_bf16 = ml_dtypes.bfloat16

if "/opt/trn_rl_repo" not in sys.path:
    sys.path.insert(0, "/opt/trn_rl_repo")

B = 4
N = 2048
DIM = 768
HEADS = 12
DHEAD = 64
SCALE = DIM ** (-0.5)
NCORES = 8
HPC = 6  # heads per core
FEAT = HPC * DHEAD  # 384 per-core attention features

_PROGRAM = None  # (nc,) cached compiled bass program


def _build_program():
    from contextlib import ExitStack

    import concourse.bass as bass
    import concourse.tile as tile
    from concourse import bacc, mybir
    from concourse.masks import make_identity

    f32 = mybir.dt.float32
    f32r = mybir.dt.float32r
    bf16 = mybir.dt.bfloat16
    f8 = mybir.dt.float8e4
    DRmode = mybir.MatmulPerfMode.DoubleRow
    Alu = mybir.AluOpType
    ActF = mybir.ActivationFunctionType

    nc = bacc.Bacc("TRN2", target_bir_lowering=False, debug=False)

    xT_in = nc.dram_tensor("xT", [DIM, N], bf16, kind="ExternalInput")
    wqkvT = nc.dram_tensor("wqkvT", [DIM, 3 * FEAT], bf16, kind="ExternalInput")
    bqkv = nc.dram_tensor("bqkv", [128, 9], f32, kind="ExternalInput")
    woutT = nc.dram_tensor("woutT", [FEAT, DIM], f32r, kind="ExternalInput")
    out_T = nc.dram_tensor("outT", [DIM, N], bf16, kind="ExternalOutput")

    NT = N // 128  # 16 n-tiles
    KC = DIM // 128  # 6 contraction chunks for dim
    NSPAN = N // 512  # 4 moving spans

    with tile.TileContext(nc) as tc, ExitStack() as ctx:
        const = ctx.enter_context(tc.tile_pool(name="const", bufs=1))
        identity_bf = const.tile([128, 128], bf16)
        make_identity(nc, identity_bf)
        ones_f32 = const.tile([128, 1], f32)
        nc.vector.memset(ones_f32[:, :], 1.0)
        ones65 = const.tile([65, 128], f32r)
        nc.vector.tensor_copy(
            out=ones65[:, :], in_=ones_f32[0:65, :].to_broadcast((65, 128))
        )
        bias_sb = const.tile([128, 9], f32)

        # ---- Phase 1: staged loads ordered by first-consumer ----
        # chain order consumes w cols m*128 for m = 0 (q), 3 (k), 6 (v) of
        # pair 0 first, and xT spans left-to-right; load those slices first,
        # fanned across the gpsimd/sync/vector DMA queues.
        xt_pool = ctx.enter_context(tc.tile_pool(name="xT", bufs=1))
        xT = xt_pool.tile([128, KC, N], bf16)
        wpool = ctx.enter_context(tc.tile_pool(name="w", bufs=1))
        w_all = wpool.tile([128, KC, 3 * FEAT], bf16)

        def ld_w(eng, j, c0, c1):
            eng.dma_start(w_all[:, j, c0:c1], wqkvT[j * 128 : (j + 1) * 128, c0:c1])

        def ld_x(eng, j, c0, c1):
            eng.dma_start(xT[:, j, c0:c1], xT_in[j * 128 : (j + 1) * 128, c0:c1])

        nc.scalar.dma_start(bias_sb[:, :], bqkv[:, :])
        for j in range(KC):
            ld_x(nc.gpsimd, j, 0, 512)
            ld_w(nc.sync, j, 0, 128)
            ld_w(nc.scalar, j, 768, 896)
        for j in range(KC):
            ld_x(nc.gpsimd, j, 512, 1024)
            ld_w(nc.sync, j, 384, 512)
        for j in range(KC):
            ld_x(nc.gpsimd, j, 1024, 1536)
            ld_x(nc.sync, j, 1536, 2048)
        for j in range(KC):
            ld_w(nc.sync, j, 128, 384)
            ld_w(nc.gpsimd, j, 896, 1152)
        for j in range(KC):
            ld_w(nc.sync, j, 512, 768)
        wout_sb = wpool.tile([128, 3, DIM], f32r)
        for c in range(3):
            nc.sync.dma_start(wout_sb[:, c, :], woutT[c * 128 : (c + 1) * 128, :])

        # PSUM: spool 3x[128,2,512] (6 banks) + opool 2x[128,512] (2 banks)
        spool = ctx.enter_context(tc.tile_pool(name="spsum", bufs=3, space="PSUM"))
        opool = ctx.enter_context(tc.tile_pool(name="opsum", bufs=2, space="PSUM"))

        # warm the PE's HAM clock-gate while the first DMAs land: ~3us of
        # dummy matmuls on the identity tile move the 1.2->2.4GHz transition
        # ahead of the first real chains
        warm = spool.tile([128, 2, 512], f32, tag="s", name="warm")
        for _ in range(28):
            nc.tensor.matmul(
                warm[:, 0, 0:128],
                identity_bf[:, :],
                identity_bf[:, :],
                start=True,
                stop=True,
            )

        qk_pool = ctx.enter_context(tc.tile_pool(name="qk", bufs=2))
        va_pool = ctx.enter_context(tc.tile_pool(name="vaug", bufs=1))
        # v for ALL pairs, produced directly in [keys, vfeat] layout (no PE
        # transposes): per kc block and pair hp:
        #   j0 = [vA@0:64 | ones@64 | 0]     -> attn@v values on partitions
        #        0:64, denom on 64
        #   j1 = [ones@128 | 0 | vB@192:256] -> values on partitions 64:128,
        #        denom on 0
        vaug_all = va_pool.tile([128, NT, 3, 256], bf16)
        exp_pool = ctx.enter_context(tc.tile_pool(name="expT", bufs=6))
        rcp_pool = ctx.enter_context(tc.tile_pool(name="rcp", bufs=2))
        sbo_pool = ctx.enter_context(tc.tile_pool(name="sbo", bufs=3))
        ao_pool = ctx.enter_context(tc.tile_pool(name="attnout", bufs=1))
        attn_outT = ao_pool.tile([128, 3, N], f32r)
        ost_pool = ctx.enter_context(tc.tile_pool(name="ostage", bufs=3))

        def emit_qk_chain(qk_t, hp, idx, m, sp2):
            """One (m, span-pair): 12 matmuls + biased copyback."""
            ps = spool.tile([128, 2, 512], f32, tag="s", name="ps_qkv")
            for u in range(2):
                span = 2 * sp2 + u
                for j in range(KC):
                    nc.tensor.matmul(
                        ps[:, u, :],
                        w_all[:, j, m * 128 : (m + 1) * 128],
                        xT[:, j, span * 512 : (span + 1) * 512],
                        start=(j == 0),
                        stop=(j == KC - 1),
                    )
            cols = slice(sp2 * 1024, (sp2 + 1) * 1024)
            # head A -> chunk idx rows 0:64; head B -> chunk idx+2 rows 64:128
            nc.vector.tensor_scalar(
                qk_t[0:64, idx, cols].rearrange("p (a b) -> p a b", a=2),
                ps[0:64, :, :],
                bias_sb[0:64, m : m + 1],
                None,
                Alu.add,
            )
            nc.vector.tensor_scalar(
                qk_t[64:128, idx + 2, cols].rearrange("p (a b) -> p a b", a=2),
                ps[64:128, :, :],
                bias_sb[64:128, m : m + 1],
                None,
                Alu.add,
            )

        def vinit():
            # pair 0's zeros go on the scalar queue (free early — gpsimd is
            # busy generating load descriptors) so the DVE ones-copies, which
            # sit at the head of the in-order DVE queue, unblock before the
            # first q/k copybacks need the DVE. v bias is folded into b_out
            # on the host (softmax rows sum to 1), so no bias add here.
            nc.scalar.memzero(vaug_all[:, :, 0, :])
            nc.vector.tensor_copy(
                out=vaug_all[:, :, 0, 64:65],
                in_=ones_f32[:, :].to_broadcast((128, NT, 1)),
            )
            nc.vector.tensor_copy(
                out=vaug_all[:, :, 0, 128:129],
                in_=ones_f32[:, :].to_broadcast((128, NT, 1)),
            )

        def vinit_rest():
            # pairs 1-2: not needed until their attn@v, well past startup
            nc.gpsimd.memset(vaug_all[:, :, 1:3, :], 0.0)
            nc.vector.tensor_copy(
                out=vaug_all[:, :, 1:3, 64:65],
                in_=ones_f32[:, :].to_broadcast((128, NT, 2, 1)),
            )
            nc.vector.tensor_copy(
                out=vaug_all[:, :, 1:3, 128:129],
                in_=ones_f32[:, :].to_broadcast((128, NT, 2, 1)),
            )

        def emit_vunit(kc):
            """v for one 128-key block, all 3 pairs: x.T @ w_v directly in
            [keys, vfeat] layout — no transposes needed."""
            pv = spool.tile([128, 384], f32, tag="s", name="pv")
            for j in range(KC):
                nc.tensor.matmul(
                    pv[:, :],
                    xT[:, j, kc * 128 : (kc + 1) * 128],
                    w_all[:, j, 768:1152],
                    start=(j == 0),
                    stop=(j == KC - 1),
                )
            pvv = pv[:, :].rearrange("p (h t c) -> p h t c", h=3, t=2)
            nc.vector.tensor_copy(out=vaug_all[:, kc, :, 0:64], in_=pvv[:, :, 0, :])
            nc.vector.tensor_copy(
                out=vaug_all[:, kc, :, 192:256], in_=pvv[:, :, 1, :]
            )

        def make_pair_units(hp):
            """Allocate q/k tile + return (tile, list of PE filler closures)."""
            qk_t = qk_pool.tile([128, 4, N], bf16, name="qk", tag="qk")
            units = []

            def zero_pads():
                # zero the unused halves so K/M padding contributes nothing
                nc.gpsimd.memset(qk_t[64:128, 0:2, :], 0.0)
                nc.gpsimd.memset(qk_t[0:64, 2:4, :], 0.0)

            units.append(zero_pads)
            # k chains (idx 1) for BOTH span-pairs before the second q chain:
            # the first score block contracts over all 2048 keys
            order = [
                (0, hp, 0),
                (1, 3 + hp, 0),
                (1, 3 + hp, 1),
                (0, hp, 1),
            ]
            for idx, m, sp2 in order:
                units.append(
                    lambda i=idx, mm=m, s=sp2: emit_qk_chain(qk_t, hp, i, mm, s)
                )
            return qk_t, units

        def emit_outproj(m, span):
            """One out-proj tile [128, 512]: 3 matmuls + copyback + DMA."""
            ps = spool.tile([128, 2, 512], f32, tag="s", name="ps_op")
            for c in range(3):
                nc.tensor.matmul(
                    ps[:, 0, :],
                    wout_sb[:, c, m * 128 : (m + 1) * 128],
                    attn_outT[:, c, span * 512 : (span + 1) * 512],
                    start=(c == 0),
                    stop=(c == 2),
                )
            ostage = ost_pool.tile([128, 512], bf16, name="ostage", tag="ostage")
            nc.vector.tensor_copy(out=ostage[:, :], in_=ps[:, 0, :])
            eng = (nc.gpsimd, nc.sync, nc.scalar)[m % 3]
            eng.dma_start(
                out_T[m * 128 : (m + 1) * 128, span * 512 : (span + 1) * 512],
                ostage[:, :],
            )

        # ---- phase 1: q/k for pair 0 (DMA-gated). v units are emitted
        # just-in-time inside pair 0's first span (2 per half) so each
        # vunit(kc) precedes the attn@v that reads it in program order ----
        vinit()
        vq = [lambda k=kc: emit_vunit(k) for kc in range(NT)]
        cur_qk, units0 = make_pair_units(0)
        for u in units0[:4]:
            u()
        vinit_rest()

        # ---- attention per head pair, interleaving filler PE work ----
        for hp in range(3):
            qk = cur_qk
            if hp < 2:
                cur_qk, nxt = make_pair_units(hp + 1)
                filler = (units0[4:] + nxt) if hp == 0 else nxt
                fill_stride = max(1, (64 + len(filler)) // (len(filler) + 1))
            else:
                filler = []  # outproj units appended dynamically by normalize
                fill_stride = 1
            half_ctr = 0
            pending = [None]  # deferred normalize closure

            def flush_pending():
                if pending[0] is not None:
                    pending[0]()
                    pending[0] = None

            for j in range(2):
                qT = qk[:, 2 * j, :]
                kT = qk[:, 2 * j + 1, :]
                for span in range(NSPAN):
                    po = opool.tile([128, 512], f32, tag="o", name="po")
                    ets = []
                    for half in range(8):
                        ps = spool.tile([128, 2, 512], f32, tag="s", name="ps_s")
                        for u in range(2):
                            kc = 2 * half + u
                            nc.tensor.matmul(
                                ps[:, u, :],
                                kT[:, kc * 128 : (kc + 1) * 128],
                                qT[:, span * 512 : (span + 1) * 512],
                                start=True,
                                stop=True,
                            )
                        et = exp_pool.tile([128, 2, 512], bf16)
                        nc.scalar.activation(
                            et[:, :, :], ps[:, :, :], ActF.Exp, scale=float(SCALE)
                        )
                        ets.append(et)
                        if half == 5:
                            # deferred so the DVE recip chain has finished —
                            # the PE broadcast below never stalls
                            flush_pending()
                        if vq:
                            # pair 0, first span only: keep vunit(kc) ahead
                            # of the attn@v that consumes it
                            vq.pop(0)()
                            if vq:
                                vq.pop(0)()
                        if half >= 1:
                            pet = ets[half - 1]
                            for u in range(2):
                                kc = 2 * (half - 1) + u
                                nc.tensor.matmul(
                                    po[:, :],
                                    vaug_all[:, kc, hp, j * 128 : (j + 1) * 128],
                                    pet[:, u, :],
                                    start=(kc == 0),
                                    stop=False,
                                )
                        half_ctr += 1
                        if hp == 2:
                            if len(filler) > 6:
                                filler.pop(0)()
                        elif filler and fill_stride and half_ctr % fill_stride == 0:
                            filler.pop(0)()
                    pet = ets[7]
                    for u in range(2):
                        kc = 14 + u
                        nc.tensor.matmul(
                            po[:, :],
                            vaug_all[:, kc, hp, j * 128 : (j + 1) * 128],
                            pet[:, u, :],
                            start=False,
                            stop=(kc == 15),
                        )
                    # denominator recip + PSUM->SBUF copy issued immediately
                    # so the deferred broadcast matmul never waits on DVE.
                    # j0: values on po[0:64], denom row 64; j1: denom row 0,
                    # values on po[64:128] (set by the vaug column layout) so
                    # both j write attn_outT partitions directly.
                    drow = 64 if j == 0 else 0
                    rsf = rcp_pool.tile([65, 512], f32, name="rsf", tag="rsf")
                    # NB: approx recip requires partition base 0 — rows other
                    # than drow compute garbage but are never read
                    nc.vector.reciprocal_approx_fast(rsf[0:65, :], po[0:65, :])
                    rs = rcp_pool.tile([65, 512], f32r, name="rs", tag="rs")
                    with nc.allow_low_precision(reason="fp32r round for PE bcast"):
                        nc.vector.tensor_copy(
                            out=rs[drow : drow + 1, :], in_=rsf[drow : drow + 1, :]
                        )
                    sb_o = sbo_pool.tile([128, 512], f32, name="sb_o")
                    vrows = slice(0, 64) if j == 0 else slice(64, 128)
                    nc.vector.tensor_copy(out=sb_o[vrows, :], in_=po[vrows, :])

                    def normalize(j=j, span=span, po=po, hp=hp, rs=rs, sb_o=sb_o):
                        # PE-broadcast of the recip back into po's own bank
                        # (WAR-ordered after the early copy), then divide.
                        cols = slice(span * 512, (span + 1) * 512)
                        if j == 0:
                            nc.tensor.matmul(
                                po[0:64, :],
                                ones65[64:65, 0:64],
                                rs[64:65, :],
                                start=True,
                                stop=True,
                            )
                            nc.vector.tensor_tensor(
                                out=attn_outT[0:64, hp, cols],
                                in0=sb_o[0:64, :],
                                in1=po[0:64, :],
                                op=Alu.mult,
                            )
                        else:
                            # dst must start at partition 0: broadcast the
                            # recip to all 128 partitions; rows 0:64 are junk
                            # but po's denom/zero rows are already consumed
                            nc.tensor.matmul(
                                po[:, :],
                                ones65[0:1, :],
                                rs[0:1, :],
                                start=True,
                                stop=True,
                            )
                            nc.vector.tensor_tensor(
                                out=attn_outT[64:128, hp, cols],
                                in0=sb_o[64:128, :],
                                in1=po[64:128, :],
                                op=Alu.mult,
                            )
                            if hp == 2:
                                for m in range(DIM // 128):
                                    filler.append(
                                        lambda mm=m, s=span: emit_outproj(mm, s)
                                    )

                    pending[0] = normalize
            flush_pending()
            while filler:
                filler.pop(0)()

    nc.compile()
    return nc


def _get_program():
    global _PROGRAM
    if _PROGRAM is None:
        _PROGRAM = _build_program()
    return _PROGRAM


def _round_to_f32r(a):
    """Round fp32 to the PE's fp32r format: 11-bit mantissa, low 12 bits zero
    (round to nearest, ties away handled approximately via +0x7FF + lsb)."""
    u = np.ascontiguousarray(a, dtype=np.float32).view(np.uint32)
    r = u + np.uint32(0x7FF) + ((u >> np.uint32(12)) & np.uint32(1))
    r &= np.uint32(0xFFFFF000)
    return r.view(np.float32)


def make_core_inputs(x, w_qkv, b_qkv, w_out):
    """Host-side shard: per-core input dicts for cores 0..7."""
    x = np.asarray(x, dtype=np.float32)
    w_qkv = np.asarray(w_qkv, dtype=np.float32)
    b_qkv = np.asarray(b_qkv, dtype=np.float32)
    w_out = np.asarray(w_out, dtype=np.float32)

    per_group = []
    for g in range(2):
        rows = np.concatenate(
            [
                w_qkv[qkv * DIM + g * FEAT : qkv * DIM + (g + 1) * FEAT]
                for qkv in range(3)
            ],
            axis=0,
        )  # [1152, 768]
        wqkvT_g = np.ascontiguousarray(rows.T).astype(_bf16)  # [768, 1152]
        b_rows = np.concatenate(
            [
                b_qkv[qkv * DIM + g * FEAT : qkv * DIM + (g + 1) * FEAT]
                for qkv in range(3)
            ],
            axis=0,
        )  # [1152]
        bias_g = np.ascontiguousarray(b_rows.reshape(9, 128).T)  # [128, 9]
        woutT_g = _round_to_f32r(w_out[:, g * FEAT : (g + 1) * FEAT].T)
        per_group.append((wqkvT_g, bias_g, woutT_g))

    xT_bf = [np.ascontiguousarray(x[b].T).astype(_bf16) for b in range(B)]
    in_maps = []
    for c in range(NCORES):
        b, g = c // 2, c % 2
        wqkvT_g, bias_g, woutT_g = per_group[g]
        in_maps.append(
            {
                "xT": xT_bf[b],
                "wqkvT": wqkvT_g,
                "bqkv": bias_g,
                "woutT": woutT_g,
            }
        )
    return in_maps


def assemble_output(results, b_out):
    """Host-side unshard: sum partials per batch pair, transpose, add bias."""
    b_out = np.asarray(b_out, dtype=np.float32)
    out = np.empty((B, N, DIM), dtype=np.float32)
    for b in range(B):
        pT = results[2 * b]["outT"].astype(np.float32) + results[2 * b + 1][
            "outT"
        ].astype(np.float32)  # [768, 2048]
        out[b] = pT.T + b_out[None, :]
    return out


def kernel(x, w_qkv, b_qkv, w_out, b_out):
    from concourse.bass_utils import run_bass_kernel_spmd

    nc = _get_program()
    in_maps = make_core_inputs(x, w_qkv, b_qkv, w_out)
    res = run_bass_kernel_spmd(nc, in_maps, list(range(NCORES)))
    # v bias is not applied on-device: softmax rows sum to 1, so it shifts
    # attn output by the constant b_v, i.e. adds w_out @ b_v to the output
    b_eff = np.asarray(b_out, np.float32) + np.asarray(w_out, np.float32) @ np.asarray(
        b_qkv, np.float32
    )[2 * DIM :]
    return assemble_output(res.results, b_eff)

